# revision 32
# baseline (speedup 1.0000x reference)
"""Trainium2 Bass kernel for the batched differentiable EKF.

B=8192 rows x T=2048 sequential EKF steps (2-state KF, scalar obs).
Output [B, T, 2] f32.

Design (final, device-verified 348.0us/core, rel err 4.2e-3):
- Data parallel: 1024 rows/core over 8 cores; rows -> 8 groups x 128
  partitions.
- Time parallel per core: T split into C=39 chunks of L=52 steps with a
  W=20-step warmup from a cold init (x=[z,dz], P=I). Chunk 0's warmup is
  the true filter start, so its warmup outputs are kept.
- fp16 everywhere: DVE tensor_tensor runs in 2x mode for 2-byte packed
  dtypes (0.52 ns/elem vs 1.04 f32), and fp16's 10 mantissa bits keep
  the noise floor at ~1.7e-3 rel (bf16's 1.4e-2 was too close to the
  2e-2 gate).
- Host pre-gathers inputs into the exact SBUF slab layout
  [slab][part][step][lane] (lane = group*C + chunk) so every DMA is a
  fully contiguous 128-descriptor transfer; host scatters outputs back.
- Custom DVE op EKF_R1S fuses S = pp00+scale INTO the reciprocal
  (BITWISE_NOT exponent-flip seed + one Newton pass, 6/8 uop stages),
  so the innovation-variance reciprocal is ONE instruction.
- The whole Riccati recurrence stays DVE-local so the step-to-step
  dependency never crosses engines (cross-engine recurrences stall the
  in-order queues); Pool gets only slack-tolerant ops (pq, pp11, x1');
  ACT does the bulk sigmoid/scale derivation.
- x-part lags the P-part by XDELAY=4 steps so two independent
  dependency chains keep DVE at ~92% occupancy.
- NOTE: ns (steps/slab) smaller than 12 miscomputes on real HW
  (scheduler/clock-wait issue invisible to TimelineSim) - keep ns=12.
"""

import numpy as np

import concourse.bass as bass
import concourse.bacc as bacc
import concourse.mybir as mybir
import concourse.tile as tile
from concourse.dve_ops import RECIP_APPROX_FAST_CONSTS, RECIPROCAL_APPROX_FAST
from concourse.dve_spec import Spec, Src0, Src1, C0, C1, AluOp, Bin, lower
import concourse.dve_ops as dve_ops_mod
from concourse.dve_ops import DveOp, OPS
from concourse.dve_uop import DveOpSpec


def _register_dve_op(name, spec):
    for op in OPS:
        if op.name == name:
            return op
    shas = {}
    for ver in ("v3", "v4"):
        uops = lower(spec, ver=ver)
        shas[ver] = DveOpSpec(name=name, opcode=0, uops=uops, rd1_en=True).sha(ver)
    op = DveOp(name, spec, subdim=False, uops_sha=shas)
    OPS.append(op)
    dve_ops_mod.CUSTOM_DVE_SPECS[name] = spec
    dve_ops_mod._SUB_OPCODE_FOR_NAME[name] = (
        dve_ops_mod._CUSTOM_DVE_ROW_BASE + len(OPS) - 1
    )
    assert dve_ops_mod._SUB_OPCODE_FOR_NAME[name] < 0x20
    return op


def _ref_r1s(in0, in1, c0, c1, c2):
    import numpy as np
    x = np.asarray(in0, np.float32) + np.asarray(in1, np.float32)
    not_x = (~x.view(np.int32)).view(np.float32)
    y0 = not_x * np.float32(c0)
    return (y0 * (np.float32(c1) - x * y0)).astype(np.float32)


_x = Src0 + Src1
_nx = Bin(AluOp.BITWISE_NOT, _x, _x)
_y0f = _nx * C0
R1S = _register_dve_op(
    "EKF_R1S",
    Spec(body=_y0f * (C1 - _x * _y0f), reference=_ref_r1s),
)

F16 = mybir.dt.float16
F32 = mybir.dt.float32
ALU = mybir.AluOpType
ACT = mybir.ActivationFunctionType
PART = 128

# geometry
B, T = 8192, 2048
NCORES = 8
B_LOC = B // NCORES          # 1024
G = B_LOC // PART            # 8
W, L, C = 20, 52, 39         # warmup, chunk len, chunks; C*L + W == T
GC = G * C                   # 312 lanes per partition
STEPS = W + L                # 72
NS = 12                      # steps per slab
NSLAB = STEPS // NS          # 6
XDELAY = 4

assert C * L + W == T and NSLAB * NS == STEPS


def build_core_kernel(
    ns=6,
    xdelay=4,
    io_bufs=3,
    ost_bufs=3,
    stp_bufs=3,
    uk_extra=3,
    prefetch_back=5,
    t3_eng="V",
    p11n_eng="V",
    x0_eng="V",
    k1y_eng="V",
    y_eng="V",
    x1_eng="GP",
    pq_eng="GP",
    sq01_eng="V",
    p00n_eng="V",
    u_eng="V",
    p01n_eng="V",
    drop_sq01=False,
    mq_form=False,
    uy_eng="V",
    recip_act=False,
    bulk_nibble=False,
    chain_prio=None,
    recip_fuse=False,
    geo=(20, 52, 39),
):
    W_, L_, C_ = geo
    GC_ = G * C_
    STEPS_ = W_ + L_
    assert C_ * L_ + W_ == T
    NS = ns
    NSLAB = STEPS_ // NS
    XDELAY = xdelay
    assert NSLAB * NS == STEPS_
    NSGC = NS * GC_
    nc = bacc.Bacc("TRN2", target_bir_lowering=False, debug=False)
    z_h = nc.dram_tensor("z", [NSLAB, PART, NSGC], F16, kind="ExternalInput")
    h_h = nc.dram_tensor("h", [NSLAB, PART, NSGC], F16, kind="ExternalInput")
    v_h = nc.dram_tensor("v", [NSLAB, PART, NSGC], F16, kind="ExternalInput")
    out_h = nc.dram_tensor(
        "out", [NSLAB, PART, NS * 2 * GC], F16, kind="ExternalOutput"
    )

    rc = RECIP_APPROX_FAST_CONSTS

    def dram_ap(handle, si, width):
        return bass.AP(
            tensor=handle, offset=si * PART * width, ap=[[width, PART], [1, width]]
        )

    with tile.TileContext(nc) as tc:
        with (
            tc.tile_pool(name="io", bufs=io_bufs) as iop,
            tc.tile_pool(name="ost", bufs=ost_bufs) as ostp,
            tc.tile_pool(name="st", bufs=stp_bufs) as stp,
            tc.tile_pool(name="uk", bufs=XDELAY + uk_extra) as ukp,
            tc.tile_pool(name="ini", bufs=1) as inip,
        ):
            V = nc.vector
            GP = nc.gpsimd
            SC = nc.scalar
            E = {"V": V, "GP": GP}

            p00_i = inip.tile([PART, GC], F16, tag="p00i")
            p01_i = inip.tile([PART, GC], F16, tag="p01i")
            p11_i = inip.tile([PART, GC], F16, tag="p11i")
            x0_i = inip.tile([PART, GC], F16, tag="x0i")
            x1_i = inip.tile([PART, GC], F16, tag="x1i")
            bias_m5 = inip.tile([PART, 1], F32, tag="bm5")
            bias_m1 = inip.tile([PART, 1], F32, tag="bm1")
            GP.memset(bias_m5[:], -5.0)
            GP.memset(bias_m1[:], -1.0)
            GP.memset(p00_i[:], 1.0)
            GP.memset(p01_i[:], 0.0)
            GP.memset(p11_i[:], 1.0)

            slab_ctx = {}
            pprev = {}
            xprev = {}
            kctx = {}

            def load_slab_dma(si):
                z_sl = iop.tile([PART, NSGC], F16, tag="z")
                h_sl = iop.tile([PART, NSGC], F16, tag="h")
                v_sl = iop.tile([PART, NSGC], F16, tag="v")
                a_sl = iop.tile([PART, NSGC], F16, tag="a")
                s_sl = iop.tile([PART, NSGC], F16, tag="scl")
                q_sl = iop.tile([PART, NSGC], F16, tag="qq")
                o_sl = ostp.tile([PART, NS * 2 * GC], F16, tag="o")
                nc.sync.dma_start(z_sl[:], dram_ap(z_h, si, NSGC))
                nc.sync.dma_start(h_sl[:], dram_ap(h_h, si, NSGC))
                nc.sync.dma_start(v_sl[:], dram_ap(v_h, si, NSGC))
                slab_ctx[si] = d = dict(
                    h_sl=h_sl, v_sl=v_sl, a_sl=a_sl, s_sl=s_sl, q_sl=q_sl,
                    zv=z_sl[:].rearrange("p (s gc) -> p s gc", s=NS),
                    av=a_sl[:].rearrange("p (s gc) -> p s gc", s=NS),
                    sv=s_sl[:].rearrange("p (s gc) -> p s gc", s=NS),
                    qv=q_sl[:].rearrange("p (s gc) -> p s gc", s=NS),
                    o_sl=o_sl,
                    ov=o_sl[:].rearrange(
                        "p (s two gc) -> p s two gc", s=NS, two=2
                    ),
                )
                if not bulk_nibble:
                    nk = NSGC // 4
                    for ci in range(4):
                        emit_bulk(si, slice(ci * nk, (ci + 1) * nk))
                return slab_ctx[si]

            def _unused_emit_bulk_hook(si, rng):
                pass

            def emit_bulk(si, rng):
                """ACT derivation for one range of slab si:
                a = 0.5 + 0.5*sigmoid(10h-5); t = relu(100v-1);
                qq = 0.1t + 0.1; scl = t + 1."""
                sl = slab_ctx[si]
                cs = rng
                h_sl, v_sl = sl["h_sl"], sl["v_sl"]
                a_sl, s_sl, q_sl = sl["a_sl"], sl["s_sl"], sl["q_sl"]
                SC.activation(
                    a_sl[:][:, cs], h_sl[:][:, cs], ACT.Sigmoid,
                    bias=bias_m5[:], scale=10.0,
                )
                SC.activation(
                    a_sl[:][:, cs], a_sl[:][:, cs], ACT.Copy,
                    bias=0.5, scale=0.5,
                )
                SC.activation(
                    s_sl[:][:, cs], v_sl[:][:, cs], ACT.Relu,
                    bias=bias_m1[:], scale=100.0,
                )
                SC.activation(
                    q_sl[:][:, cs], s_sl[:][:, cs], ACT.Copy,
                    bias=0.1, scale=0.1,
                )
                SC.activation(
                    s_sl[:][:, cs], s_sl[:][:, cs], ACT.Copy,
                    bias=1.0, scale=1.0,
                )

            def emit_p_head(gs):
                si, s = divmod(gs, NS)
                sl = slab_ctx[si]
                A = sl["av"][:, s]
                QQ = sl["qv"][:, s]
                p00p, p01p, p11p = pprev["p00"], pprev["p01"], pprev["p11"]

                pp11 = stp.tile([PART, GC], F16, tag="pp11")
                if mq_form:
                    pq = None
                else:
                    pq = stp.tile([PART, GC], F16, tag="pq")
                    E[pq_eng].tensor_tensor(
                        out=pq[:], in0=QQ, in1=p00p, op=ALU.add
                    )
                E[pq_eng].tensor_tensor(out=pp11[:], in0=QQ, in1=p11p, op=ALU.add)

                t1 = stp.tile([PART, GC], F16, tag="t1")
                pp01 = stp.tile([PART, GC], F16, tag="pp01")
                g2 = stp.tile([PART, GC], F16, tag="g2")
                m = stp.tile([PART, GC], F16, tag="m")
                import contextlib
                hp = (tc.high_priority(offset=chain_prio)
                      if chain_prio else contextlib.nullcontext())
                with hp:
                    V.tensor_tensor(out=t1[:], in0=A, in1=p11p, op=ALU.mult)
                    V.tensor_tensor(
                        out=pp01[:], in0=p01p, in1=t1[:], op=ALU.add
                    )
                    V.tensor_tensor(
                        out=g2[:], in0=pp01[:], in1=p01p, op=ALU.add
                    )
                    V.tensor_tensor(out=m[:], in0=A, in1=g2[:], op=ALU.mult)
                return dict(sl=sl, s=s, pq=pq, pp11=pp11, pp01=pp01, m=m,
                            p00p=p00p)

            def emit_p_tail(gs, h):
                import contextlib
                hp = (tc.high_priority(offset=chain_prio)
                      if chain_prio else contextlib.nullcontext())
                sl, s = h["sl"], h["s"]
                SCL = sl["sv"][:, s]
                pq, pp11, pp01, m = (
                    h["pq"], h["pp11"], h["pp01"], h["m"]
                )
                pp00 = stp.tile([PART, GC], F16, tag="pp00")
                S = (None if recip_fuse
                     else stp.tile([PART, GC], F16, tag="S"))
                r = stp.tile([PART, GC], F16, tag="r")
                u = ukp.tile([PART, GC], F16, tag="u")
                k1 = ukp.tile([PART, GC], F16, tag="k1")
                p00n = stp.tile([PART, GC], F16, tag="p00")
                p01n = stp.tile([PART, GC], F16, tag="p01")
                t3 = stp.tile([PART, GC], F16, tag="t3")
                p11n = stp.tile([PART, GC], F16, tag="p11")
                with hp:
                    if mq_form:
                        QQ = sl["qv"][:, s]
                        mq = stp.tile([PART, GC], F16, tag="mq")
                        V.tensor_tensor(
                            out=mq[:], in0=m[:], in1=QQ, op=ALU.add
                        )
                        V.tensor_tensor(
                            out=pp00[:], in0=mq[:], in1=h["p00p"], op=ALU.add
                        )
                    else:
                        V.tensor_tensor(
                            out=pp00[:], in0=pq[:], in1=m[:], op=ALU.add
                        )
                    if not recip_fuse:
                        V.tensor_tensor(
                            out=S[:], in0=pp00[:], in1=SCL, op=ALU.add
                        )
                if recip_fuse:
                    V._custom_dve(
                        R1S, out=r[:], in0=pp00[:], in1=SCL,
                        s0=rc["s0"], s1=rc["s1"],
                    )
                elif recip_act:
                    SC.add_instruction(mybir.InstActivation(
                        name=nc.get_next_instruction_name(),
                        func=ACT.Reciprocal,
                        ins=[
                            SC.lower_ap(S[:]),
                            mybir.ImmediateValue(dtype=F32, value=0.0),
                            mybir.ImmediateValue(dtype=F32, value=1.0),
                            mybir.ImmediateValue(dtype=F32, value=0.0),
                        ],
                        outs=[SC.lower_ap(r[:])],
                    ))
                else:
                    V._custom_dve(
                        RECIPROCAL_APPROX_FAST, out=r[:], in0=S[:],
                        s0=rc["s0"], s1=rc["s1"], imm2=rc["imm2"],
                    )
                E[u_eng].tensor_tensor(
                    out=u[:], in0=SCL, in1=r[:], op=ALU.mult
                )
                V.tensor_tensor(out=k1[:], in0=r[:], in1=pp01[:], op=ALU.mult)
                E[p00n_eng].tensor_tensor(
                    out=p00n[:], in0=u[:], in1=pp00[:], op=ALU.mult
                )
                E[p01n_eng].tensor_tensor(
                    out=p01n[:], in0=SCL, in1=k1[:], op=ALU.mult
                )
                if drop_sq01:
                    E[t3_eng].tensor_tensor(
                        out=t3[:], in0=k1[:], in1=pp01[:], op=ALU.mult
                    )
                else:
                    sq01 = stp.tile([PART, GC], F16, tag="sq01")
                    E[sq01_eng].tensor_tensor(
                        out=sq01[:], in0=pp01[:], in1=pp01[:], op=ALU.mult
                    )
                    E[t3_eng].tensor_tensor(
                        out=t3[:], in0=sq01[:], in1=r[:], op=ALU.mult
                    )
                E[p11n_eng].tensor_tensor(
                    out=p11n[:], in0=pp11[:], in1=t3[:], op=ALU.subtract
                )
                pprev.update(p00=p00n[:], p01=p01n[:], p11=p11n[:])
                kctx[gs] = (u, k1)

            def emit_x(gs):
                si, s = divmod(gs, NS)
                sl = slab_ctx[si]
                Z = sl["zv"][:, s]
                A = sl["av"][:, s]
                ov = sl["ov"]
                u, k1 = kctx.pop(gs)
                x0p, x1p = xprev["x0"], xprev["x1"]

                t4 = stp.tile([PART, GC], F16, tag="t4")
                xp = stp.tile([PART, GC], F16, tag="xp")
                y = stp.tile([PART, GC], F16, tag="y")
                uy = stp.tile([PART, GC], F16, tag="uy")
                k1y = stp.tile([PART, GC], F16, tag="k1y")
                V.tensor_tensor(out=t4[:], in0=A, in1=x1p, op=ALU.mult)
                V.tensor_tensor(out=xp[:], in0=x0p, in1=t4[:], op=ALU.add)
                E[y_eng].tensor_tensor(
                    out=y[:], in0=Z, in1=xp[:], op=ALU.subtract
                )
                E[uy_eng].tensor_tensor(
                    out=uy[:], in0=u[:], in1=y[:], op=ALU.mult
                )
                E[x0_eng].tensor_tensor(
                    out=ov[:, s, 0], in0=Z, in1=uy[:], op=ALU.subtract
                )
                E[k1y_eng].tensor_tensor(
                    out=k1y[:], in0=k1[:], in1=y[:], op=ALU.mult
                )
                E[x1_eng].tensor_tensor(
                    out=ov[:, s, 1], in0=x1p, in1=k1y[:], op=ALU.add
                )
                xprev.update(x0=ov[:, s, 0], x1=ov[:, s, 1])

                if s == NS - 1:
                    nc.sync.dma_start(
                        dram_ap(out_h, si, NS * 2 * GC), sl["o_sl"][:]
                    )

            for gs in range(STEPS + XDELAY):
                if gs < STEPS:
                    si, s = divmod(gs, NS)
                    if gs == 0:
                        ctx = load_slab_dma(0)
                        if bulk_nibble:
                            nkr0 = NSGC // NS
                            emit_bulk(0, slice(0, 2 * nkr0))
                        zv = ctx["zv"]
                        V.tensor_copy(x0_i[:], zv[:, 0])
                        V.tensor_tensor(
                            out=x1_i[:], in0=zv[:, 1], in1=zv[:, 0],
                            op=ALU.subtract,
                        )
                        pprev.update(
                            p00=p00_i[:], p01=p01_i[:], p11=p11_i[:]
                        )
                        xprev.update(x0=x0_i[:], x1=x1_i[:])
                    if s == max(0, NS - prefetch_back) and si + 1 < NSLAB:
                        load_slab_dma(si + 1)
                    h = emit_p_head(gs)
                    if gs - XDELAY >= 0:
                        emit_x(gs - XDELAY)
                    emit_p_tail(gs, h)
                    if bulk_nibble:
                        nkr = NSGC // NS
                        if si == 0 and 0 <= gs < NS - 2:
                            emit_bulk(0, slice((gs + 2) * nkr, (gs + 3) * nkr))
                        pf = max(0, NS - prefetch_back)
                        if si + 1 < NSLAB and s >= pf:
                            left = NS - pf
                            r0 = (s - pf) * NS // left
                            r1 = (s - pf + 1) * NS // left
                            if r1 > r0:
                                emit_bulk(
                                    si + 1, slice(r0 * nkr, r1 * nkr)
                                )
                else:
                    emit_x(gs - XDELAY)
    nc.compile()
    return nc


_nc_cache = {}


def _get_nc():
    if "nc" not in _nc_cache:
        # best verified config: chain-local Riccati on DVE + fused-S
        # 1-Newton reciprocal custom op; 348.0us/core, rel err 4.2e-3
        _nc_cache["nc"] = build_core_kernel(
            ns=12, io_bufs=2, recip_fuse=True
        )
    return _nc_cache["nc"]


# host-side gather/scatter index: col of (step gs, chunk c) = c*L + gs
_COLS = (np.arange(C)[None, :] * L + np.arange(STEPS)[:, None])  # [STEPS, C]


def _stage_input(arr):
    """[B_LOC, T] f32 -> [NSLAB, PART, NS*GC] f16 in slab layout."""
    xf = arr.astype(np.float16)
    g3 = xf.reshape(G, PART, T)[:, :, _COLS]          # [G, P, STEPS, C]
    g4 = np.transpose(g3, (2, 1, 0, 3))                # [STEPS, P, G, C]
    st = g4.reshape(NSLAB, NS, PART, GC)
    st = np.transpose(st, (0, 2, 1, 3)).reshape(NSLAB, PART, NS * GC)
    return np.ascontiguousarray(st)


def _unstage_output(dev_out):
    """[NSLAB, PART, NS*2*GC] f16 -> [B_LOC, T, 2] f32."""
    o = np.asarray(dev_out).reshape(NSLAB, PART, NS, 2, G, C)
    o = np.transpose(o, (4, 1, 0, 2, 5, 3))            # [G, P, NSLAB, NS, C, 2]
    o = o.reshape(B_LOC, STEPS, C, 2).astype(np.float32)
    res = np.empty((B_LOC, T, 2), np.float32)
    res[:, _COLS[:, 0], :] = o[:, :, 0, :]
    res[:, _COLS[W:, 1:], :] = o[:, W:, 1:, :]
    return res


def kernel(price: np.ndarray, hurst: np.ndarray, vol_sigma: np.ndarray) -> np.ndarray:
    from concourse import bass_utils

    price = np.ascontiguousarray(price, dtype=np.float32)
    hurst = np.ascontiguousarray(hurst, dtype=np.float32)
    vol_sigma = np.ascontiguousarray(vol_sigma, dtype=np.float32)
    nc = _get_nc()
    in_maps = []
    for k in range(NCORES):
        sl = slice(k * B_LOC, (k + 1) * B_LOC)
        in_maps.append(
            {
                "z": _stage_input(price[sl]),
                "h": _stage_input(hurst[sl]),
                "v": _stage_input(vol_sigma[sl]),
            }
        )
    res = bass_utils.run_bass_kernel_spmd(
        nc, in_maps, core_ids=list(range(NCORES))
    )
    return np.concatenate(
        [_unstage_output(r["out"]) for r in res.results], axis=0
    )


# revision 34
# speedup vs baseline: 1.0193x; 1.0193x over previous
"""Trainium2 Bass kernel for the batched differentiable EKF.

B=8192 rows x T=2048 sequential EKF steps (2-state KF, scalar obs).
Output [B, T, 2] f32.

Design (final, device-verified 348.0us/core, rel err 4.2e-3):
- Data parallel: 1024 rows/core over 8 cores; rows -> 8 groups x 128
  partitions.
- Time parallel per core: T split into C=39 chunks of L=52 steps with a
  W=20-step warmup from a cold init (x=[z,dz], P=I). Chunk 0's warmup is
  the true filter start, so its warmup outputs are kept.
- fp16 everywhere: DVE tensor_tensor runs in 2x mode for 2-byte packed
  dtypes (0.52 ns/elem vs 1.04 f32), and fp16's 10 mantissa bits keep
  the noise floor at ~1.7e-3 rel (bf16's 1.4e-2 was too close to the
  2e-2 gate).
- Host pre-gathers inputs into the exact SBUF slab layout
  [slab][part][step][lane] (lane = group*C + chunk) so every DMA is a
  fully contiguous 128-descriptor transfer; host scatters outputs back.
- Custom DVE op EKF_R1S fuses S = pp00+scale INTO the reciprocal
  (BITWISE_NOT exponent-flip seed + one Newton pass, 6/8 uop stages),
  so the innovation-variance reciprocal is ONE instruction.
- The whole Riccati recurrence stays DVE-local so the step-to-step
  dependency never crosses engines (cross-engine recurrences stall the
  in-order queues); Pool gets only slack-tolerant ops (pq, pp11, x1');
  ACT does the bulk sigmoid/scale derivation.
- x-part lags the P-part by XDELAY=4 steps so two independent
  dependency chains keep DVE at ~92% occupancy.
- NOTE: ns (steps/slab) smaller than 12 miscomputes on real HW
  (scheduler/clock-wait issue invisible to TimelineSim) - keep ns=12.
"""

import numpy as np

import concourse.bass as bass
import concourse.bacc as bacc
import concourse.mybir as mybir
import concourse.tile as tile
from concourse.dve_ops import RECIP_APPROX_FAST_CONSTS, RECIPROCAL_APPROX_FAST
from concourse.dve_spec import Spec, Src0, Src1, C0, C1, AluOp, Bin, lower
import concourse.dve_ops as dve_ops_mod
from concourse.dve_ops import DveOp, OPS
from concourse.dve_uop import DveOpSpec


def _register_dve_op(name, spec):
    for op in OPS:
        if op.name == name:
            return op
    shas = {}
    for ver in ("v3", "v4"):
        uops = lower(spec, ver=ver)
        shas[ver] = DveOpSpec(name=name, opcode=0, uops=uops, rd1_en=True).sha(ver)
    op = DveOp(name, spec, subdim=False, uops_sha=shas)
    OPS.append(op)
    dve_ops_mod.CUSTOM_DVE_SPECS[name] = spec
    dve_ops_mod._SUB_OPCODE_FOR_NAME[name] = (
        dve_ops_mod._CUSTOM_DVE_ROW_BASE + len(OPS) - 1
    )
    assert dve_ops_mod._SUB_OPCODE_FOR_NAME[name] < 0x20
    return op


def _ref_r1s(in0, in1, c0, c1, c2):
    import numpy as np
    x = np.asarray(in0, np.float32) + np.asarray(in1, np.float32)
    not_x = (~x.view(np.int32)).view(np.float32)
    y0 = not_x * np.float32(c0)
    return (y0 * (np.float32(c1) - x * y0)).astype(np.float32)


_x = Src0 + Src1
_nx = Bin(AluOp.BITWISE_NOT, _x, _x)
_y0f = _nx * C0
R1S = _register_dve_op(
    "EKF_R1S",
    Spec(body=_y0f * (C1 - _x * _y0f), reference=_ref_r1s),
)

F16 = mybir.dt.float16
F32 = mybir.dt.float32
ALU = mybir.AluOpType
ACT = mybir.ActivationFunctionType
PART = 128

# geometry
B, T = 8192, 2048
NCORES = 8
B_LOC = B // NCORES          # 1024
G = B_LOC // PART            # 8
W, L, C = 20, 52, 39         # warmup, chunk len, chunks; C*L + W == T
GC = G * C                   # 312 lanes per partition
STEPS = W + L                # 72
NS = 12                      # steps per slab
NSLAB = STEPS // NS          # 6
XDELAY = 4

assert C * L + W == T and NSLAB * NS == STEPS


def build_core_kernel(
    ns=6,
    xdelay=4,
    io_bufs=3,
    ost_bufs=3,
    stp_bufs=3,
    uk_extra=3,
    prefetch_back=5,
    t3_eng="V",
    p11n_eng="V",
    x0_eng="V",
    k1y_eng="V",
    y_eng="V",
    x1_eng="GP",
    pq_eng="GP",
    sq01_eng="V",
    p00n_eng="V",
    u_eng="V",
    p01n_eng="V",
    drop_sq01=False,
    mq_form=False,
    uy_eng="V",
    recip_act=False,
    bulk_nibble=False,
    chain_prio=None,
    recip_fuse=False,
    geo=(20, 52, 39),
):
    W_, L_, C_ = geo
    GC_ = G * C_
    STEPS_ = W_ + L_
    assert C_ * L_ + W_ == T
    NS = ns
    NSLAB = STEPS_ // NS
    XDELAY = xdelay
    assert NSLAB * NS == STEPS_
    NSGC = NS * GC_
    nc = bacc.Bacc("TRN2", target_bir_lowering=False, debug=False)
    z_h = nc.dram_tensor("z", [NSLAB, PART, NSGC], F16, kind="ExternalInput")
    h_h = nc.dram_tensor("h", [NSLAB, PART, NSGC], F16, kind="ExternalInput")
    v_h = nc.dram_tensor("v", [NSLAB, PART, NSGC], F16, kind="ExternalInput")
    out_h = nc.dram_tensor(
        "out", [NSLAB, PART, NS * 2 * GC], F16, kind="ExternalOutput"
    )

    rc = RECIP_APPROX_FAST_CONSTS

    def dram_ap(handle, si, width):
        return bass.AP(
            tensor=handle, offset=si * PART * width, ap=[[width, PART], [1, width]]
        )

    with tile.TileContext(nc) as tc:
        with (
            tc.tile_pool(name="io", bufs=io_bufs) as iop,
            tc.tile_pool(name="ost", bufs=ost_bufs) as ostp,
            tc.tile_pool(name="st", bufs=stp_bufs) as stp,
            tc.tile_pool(name="uk", bufs=XDELAY + uk_extra) as ukp,
            tc.tile_pool(name="ini", bufs=1) as inip,
        ):
            V = nc.vector
            GP = nc.gpsimd
            SC = nc.scalar
            E = {"V": V, "GP": GP}

            p00_i = inip.tile([PART, GC], F16, tag="p00i")
            p01_i = inip.tile([PART, GC], F16, tag="p01i")
            p11_i = inip.tile([PART, GC], F16, tag="p11i")
            x0_i = inip.tile([PART, GC], F16, tag="x0i")
            x1_i = inip.tile([PART, GC], F16, tag="x1i")
            bias_m5 = inip.tile([PART, 1], F32, tag="bm5")
            bias_m1 = inip.tile([PART, 1], F32, tag="bm1")
            GP.memset(bias_m5[:], -5.0)
            GP.memset(bias_m1[:], -1.0)
            GP.memset(p00_i[:], 1.0)
            GP.memset(p01_i[:], 0.0)
            GP.memset(p11_i[:], 1.0)
            # preload the ACT function table off the critical path
            SC.activation(
                x0_i[:][:, 0:1], bias_m5[:], ACT.Sigmoid,
                bias=bias_m5[:], scale=1.0,
            )

            slab_ctx = {}
            pprev = {}
            xprev = {}
            kctx = {}

            def load_slab_dma(si):
                z_sl = iop.tile([PART, NSGC], F16, tag="z")
                h_sl = iop.tile([PART, NSGC], F16, tag="h")
                v_sl = iop.tile([PART, NSGC], F16, tag="v")
                a_sl = iop.tile([PART, NSGC], F16, tag="a")
                s_sl = iop.tile([PART, NSGC], F16, tag="scl")
                q_sl = iop.tile([PART, NSGC], F16, tag="qq")
                o_sl = ostp.tile([PART, NS * 2 * GC], F16, tag="o")
                nc.sync.dma_start(h_sl[:], dram_ap(h_h, si, NSGC))
                nc.sync.dma_start(v_sl[:], dram_ap(v_h, si, NSGC))
                nc.sync.dma_start(z_sl[:], dram_ap(z_h, si, NSGC))
                slab_ctx[si] = d = dict(
                    h_sl=h_sl, v_sl=v_sl, a_sl=a_sl, s_sl=s_sl, q_sl=q_sl,
                    zv=z_sl[:].rearrange("p (s gc) -> p s gc", s=NS),
                    av=a_sl[:].rearrange("p (s gc) -> p s gc", s=NS),
                    sv=s_sl[:].rearrange("p (s gc) -> p s gc", s=NS),
                    qv=q_sl[:].rearrange("p (s gc) -> p s gc", s=NS),
                    o_sl=o_sl,
                    ov=o_sl[:].rearrange(
                        "p (s two gc) -> p s two gc", s=NS, two=2
                    ),
                )
                if not bulk_nibble:
                    if si == 0:
                        bnds = [0, 2 * GC, 5 * GC, 8 * GC, NSGC]
                    else:
                        nk = NSGC // 4
                        bnds = [0, nk, 2 * nk, 3 * nk, NSGC]
                    for ci in range(4):
                        emit_bulk(si, slice(bnds[ci], bnds[ci + 1]))
                return slab_ctx[si]

            def _unused_emit_bulk_hook(si, rng):
                pass

            def emit_bulk(si, rng):
                """ACT derivation for one range of slab si:
                a = 0.5 + 0.5*sigmoid(10h-5); t = relu(100v-1);
                qq = 0.1t + 0.1; scl = t + 1."""
                sl = slab_ctx[si]
                cs = rng
                h_sl, v_sl = sl["h_sl"], sl["v_sl"]
                a_sl, s_sl, q_sl = sl["a_sl"], sl["s_sl"], sl["q_sl"]
                SC.activation(
                    a_sl[:][:, cs], h_sl[:][:, cs], ACT.Sigmoid,
                    bias=bias_m5[:], scale=10.0,
                )
                SC.activation(
                    a_sl[:][:, cs], a_sl[:][:, cs], ACT.Copy,
                    bias=0.5, scale=0.5,
                )
                SC.activation(
                    s_sl[:][:, cs], v_sl[:][:, cs], ACT.Relu,
                    bias=bias_m1[:], scale=100.0,
                )
                SC.activation(
                    q_sl[:][:, cs], s_sl[:][:, cs], ACT.Copy,
                    bias=0.1, scale=0.1,
                )
                SC.activation(
                    s_sl[:][:, cs], s_sl[:][:, cs], ACT.Copy,
                    bias=1.0, scale=1.0,
                )

            def emit_p_head(gs):
                si, s = divmod(gs, NS)
                sl = slab_ctx[si]
                A = sl["av"][:, s]
                QQ = sl["qv"][:, s]
                p00p, p01p, p11p = pprev["p00"], pprev["p01"], pprev["p11"]

                pp11 = stp.tile([PART, GC], F16, tag="pp11")
                if mq_form:
                    pq = None
                else:
                    pq = stp.tile([PART, GC], F16, tag="pq")
                    E[pq_eng].tensor_tensor(
                        out=pq[:], in0=QQ, in1=p00p, op=ALU.add
                    )
                E[pq_eng].tensor_tensor(out=pp11[:], in0=QQ, in1=p11p, op=ALU.add)

                t1 = stp.tile([PART, GC], F16, tag="t1")
                pp01 = stp.tile([PART, GC], F16, tag="pp01")
                g2 = stp.tile([PART, GC], F16, tag="g2")
                m = stp.tile([PART, GC], F16, tag="m")
                import contextlib
                hp = (tc.high_priority(offset=chain_prio)
                      if chain_prio else contextlib.nullcontext())
                with hp:
                    V.tensor_tensor(out=t1[:], in0=A, in1=p11p, op=ALU.mult)
                    V.tensor_tensor(
                        out=pp01[:], in0=p01p, in1=t1[:], op=ALU.add
                    )
                    V.tensor_tensor(
                        out=g2[:], in0=pp01[:], in1=p01p, op=ALU.add
                    )
                    V.tensor_tensor(out=m[:], in0=A, in1=g2[:], op=ALU.mult)
                return dict(sl=sl, s=s, pq=pq, pp11=pp11, pp01=pp01, m=m,
                            p00p=p00p)

            def emit_p_tail(gs, h):
                import contextlib
                hp = (tc.high_priority(offset=chain_prio)
                      if chain_prio else contextlib.nullcontext())
                sl, s = h["sl"], h["s"]
                SCL = sl["sv"][:, s]
                pq, pp11, pp01, m = (
                    h["pq"], h["pp11"], h["pp01"], h["m"]
                )
                pp00 = stp.tile([PART, GC], F16, tag="pp00")
                S = (None if recip_fuse
                     else stp.tile([PART, GC], F16, tag="S"))
                r = stp.tile([PART, GC], F16, tag="r")
                u = ukp.tile([PART, GC], F16, tag="u")
                k1 = ukp.tile([PART, GC], F16, tag="k1")
                p00n = stp.tile([PART, GC], F16, tag="p00")
                p01n = stp.tile([PART, GC], F16, tag="p01")
                t3 = stp.tile([PART, GC], F16, tag="t3")
                p11n = stp.tile([PART, GC], F16, tag="p11")
                with hp:
                    if mq_form:
                        QQ = sl["qv"][:, s]
                        mq = stp.tile([PART, GC], F16, tag="mq")
                        V.tensor_tensor(
                            out=mq[:], in0=m[:], in1=QQ, op=ALU.add
                        )
                        V.tensor_tensor(
                            out=pp00[:], in0=mq[:], in1=h["p00p"], op=ALU.add
                        )
                    else:
                        V.tensor_tensor(
                            out=pp00[:], in0=pq[:], in1=m[:], op=ALU.add
                        )
                    if not recip_fuse:
                        V.tensor_tensor(
                            out=S[:], in0=pp00[:], in1=SCL, op=ALU.add
                        )
                if recip_fuse:
                    V._custom_dve(
                        R1S, out=r[:], in0=pp00[:], in1=SCL,
                        s0=rc["s0"], s1=rc["s1"],
                    )
                elif recip_act:
                    SC.add_instruction(mybir.InstActivation(
                        name=nc.get_next_instruction_name(),
                        func=ACT.Reciprocal,
                        ins=[
                            SC.lower_ap(S[:]),
                            mybir.ImmediateValue(dtype=F32, value=0.0),
                            mybir.ImmediateValue(dtype=F32, value=1.0),
                            mybir.ImmediateValue(dtype=F32, value=0.0),
                        ],
                        outs=[SC.lower_ap(r[:])],
                    ))
                else:
                    V._custom_dve(
                        RECIPROCAL_APPROX_FAST, out=r[:], in0=S[:],
                        s0=rc["s0"], s1=rc["s1"], imm2=rc["imm2"],
                    )
                E[u_eng].tensor_tensor(
                    out=u[:], in0=SCL, in1=r[:], op=ALU.mult
                )
                V.tensor_tensor(out=k1[:], in0=r[:], in1=pp01[:], op=ALU.mult)
                E[p00n_eng].tensor_tensor(
                    out=p00n[:], in0=u[:], in1=pp00[:], op=ALU.mult
                )
                E[p01n_eng].tensor_tensor(
                    out=p01n[:], in0=SCL, in1=k1[:], op=ALU.mult
                )
                if drop_sq01:
                    E[t3_eng].tensor_tensor(
                        out=t3[:], in0=k1[:], in1=pp01[:], op=ALU.mult
                    )
                else:
                    sq01 = stp.tile([PART, GC], F16, tag="sq01")
                    E[sq01_eng].tensor_tensor(
                        out=sq01[:], in0=pp01[:], in1=pp01[:], op=ALU.mult
                    )
                    E[t3_eng].tensor_tensor(
                        out=t3[:], in0=sq01[:], in1=r[:], op=ALU.mult
                    )
                E[p11n_eng].tensor_tensor(
                    out=p11n[:], in0=pp11[:], in1=t3[:], op=ALU.subtract
                )
                pprev.update(p00=p00n[:], p01=p01n[:], p11=p11n[:])
                kctx[gs] = (u, k1)

            def emit_x(gs):
                si, s = divmod(gs, NS)
                sl = slab_ctx[si]
                Z = sl["zv"][:, s]
                A = sl["av"][:, s]
                ov = sl["ov"]
                u, k1 = kctx.pop(gs)
                x0p, x1p = xprev["x0"], xprev["x1"]

                t4 = stp.tile([PART, GC], F16, tag="t4")
                xp = stp.tile([PART, GC], F16, tag="xp")
                y = stp.tile([PART, GC], F16, tag="y")
                uy = stp.tile([PART, GC], F16, tag="uy")
                k1y = stp.tile([PART, GC], F16, tag="k1y")
                V.tensor_tensor(out=t4[:], in0=A, in1=x1p, op=ALU.mult)
                V.tensor_tensor(out=xp[:], in0=x0p, in1=t4[:], op=ALU.add)
                E[y_eng].tensor_tensor(
                    out=y[:], in0=Z, in1=xp[:], op=ALU.subtract
                )
                E[uy_eng].tensor_tensor(
                    out=uy[:], in0=u[:], in1=y[:], op=ALU.mult
                )
                E[x0_eng].tensor_tensor(
                    out=ov[:, s, 0], in0=Z, in1=uy[:], op=ALU.subtract
                )
                E[k1y_eng].tensor_tensor(
                    out=k1y[:], in0=k1[:], in1=y[:], op=ALU.mult
                )
                E[x1_eng].tensor_tensor(
                    out=ov[:, s, 1], in0=x1p, in1=k1y[:], op=ALU.add
                )
                xprev.update(x0=ov[:, s, 0], x1=ov[:, s, 1])

                if si == NSLAB - 1:
                    half = (NS // 2) * 2 * GC
                    if s == NS // 2 - 1:
                        nc.sync.dma_start(
                            bass.AP(
                                tensor=out_h,
                                offset=si * PART * NS * 2 * GC,
                                ap=[[NS * 2 * GC, PART], [1, half]],
                            ),
                            sl["o_sl"][:][:, :half],
                        )
                    elif s == NS - 1:
                        nc.sync.dma_start(
                            bass.AP(
                                tensor=out_h,
                                offset=si * PART * NS * 2 * GC + half,
                                ap=[[NS * 2 * GC, PART], [1, half]],
                            ),
                            sl["o_sl"][:][:, half:],
                        )
                elif s == NS - 1:
                    nc.sync.dma_start(
                        dram_ap(out_h, si, NS * 2 * GC), sl["o_sl"][:]
                    )

            for gs in range(STEPS + XDELAY):
                if gs < STEPS:
                    si, s = divmod(gs, NS)
                    if gs == 0:
                        ctx = load_slab_dma(0)
                        if bulk_nibble:
                            nkr0 = NSGC // NS
                            emit_bulk(0, slice(0, 2 * nkr0))
                        zv = ctx["zv"]
                        V.tensor_copy(x0_i[:], zv[:, 0])
                        V.tensor_tensor(
                            out=x1_i[:], in0=zv[:, 1], in1=zv[:, 0],
                            op=ALU.subtract,
                        )
                        pprev.update(
                            p00=p00_i[:], p01=p01_i[:], p11=p11_i[:]
                        )
                        xprev.update(x0=x0_i[:], x1=x1_i[:])
                    if s == max(0, NS - prefetch_back) and si + 1 < NSLAB:
                        load_slab_dma(si + 1)
                    h = emit_p_head(gs)
                    if gs - XDELAY >= 0:
                        emit_x(gs - XDELAY)
                    emit_p_tail(gs, h)
                    if bulk_nibble:
                        nkr = NSGC // NS
                        if si == 0 and 0 <= gs < NS - 2:
                            emit_bulk(0, slice((gs + 2) * nkr, (gs + 3) * nkr))
                        pf = max(0, NS - prefetch_back)
                        if si + 1 < NSLAB and s >= pf:
                            left = NS - pf
                            r0 = (s - pf) * NS // left
                            r1 = (s - pf + 1) * NS // left
                            if r1 > r0:
                                emit_bulk(
                                    si + 1, slice(r0 * nkr, r1 * nkr)
                                )
                else:
                    emit_x(gs - XDELAY)
    nc.compile()
    return nc


_nc_cache = {}


def _get_nc():
    if "nc" not in _nc_cache:
        # best verified config: chain-local Riccati on DVE + fused-S
        # 1-Newton reciprocal custom op; 348.0us/core, rel err 4.2e-3
        _nc_cache["nc"] = build_core_kernel(
            ns=12, io_bufs=2, recip_fuse=True
        )
    return _nc_cache["nc"]


# host-side gather/scatter index: col of (step gs, chunk c) = c*L + gs
_COLS = (np.arange(C)[None, :] * L + np.arange(STEPS)[:, None])  # [STEPS, C]


def _stage_input(arr):
    """[B_LOC, T] f32 -> [NSLAB, PART, NS*GC] f16 in slab layout."""
    xf = arr.astype(np.float16)
    g3 = xf.reshape(G, PART, T)[:, :, _COLS]          # [G, P, STEPS, C]
    g4 = np.transpose(g3, (2, 1, 0, 3))                # [STEPS, P, G, C]
    st = g4.reshape(NSLAB, NS, PART, GC)
    st = np.transpose(st, (0, 2, 1, 3)).reshape(NSLAB, PART, NS * GC)
    return np.ascontiguousarray(st)


def _unstage_output(dev_out):
    """[NSLAB, PART, NS*2*GC] f16 -> [B_LOC, T, 2] f32."""
    o = np.asarray(dev_out).reshape(NSLAB, PART, NS, 2, G, C)
    o = np.transpose(o, (4, 1, 0, 2, 5, 3))            # [G, P, NSLAB, NS, C, 2]
    o = o.reshape(B_LOC, STEPS, C, 2).astype(np.float32)
    res = np.empty((B_LOC, T, 2), np.float32)
    res[:, _COLS[:, 0], :] = o[:, :, 0, :]
    res[:, _COLS[W:, 1:], :] = o[:, W:, 1:, :]
    return res


def kernel(price: np.ndarray, hurst: np.ndarray, vol_sigma: np.ndarray) -> np.ndarray:
    from concourse import bass_utils

    price = np.ascontiguousarray(price, dtype=np.float32)
    hurst = np.ascontiguousarray(hurst, dtype=np.float32)
    vol_sigma = np.ascontiguousarray(vol_sigma, dtype=np.float32)
    nc = _get_nc()
    in_maps = []
    for k in range(NCORES):
        sl = slice(k * B_LOC, (k + 1) * B_LOC)
        in_maps.append(
            {
                "z": _stage_input(price[sl]),
                "h": _stage_input(hurst[sl]),
                "v": _stage_input(vol_sigma[sl]),
            }
        )
    res = bass_utils.run_bass_kernel_spmd(
        nc, in_maps, core_ids=list(range(NCORES))
    )
    return np.concatenate(
        [_unstage_output(r["out"]) for r in res.results], axis=0
    )


# revision 37
# speedup vs baseline: 1.0357x; 1.0161x over previous
"""Trainium2 Bass kernel for the batched differentiable EKF.

B=8192 rows x T=2048 sequential EKF steps (2-state KF, scalar obs).
Output [B, T, 2] f32.

Design (final, device-verified 341.4us/core, rel err 4.2e-3):
- Data parallel: 1024 rows/core over 8 cores; rows -> 8 groups x 128
  partitions.
- Time parallel per core: T split into C=39 chunks of L=52 steps with a
  W=20-step warmup from a cold init (x=[z,dz], P=I). Chunk 0's warmup is
  the true filter start, so its warmup outputs are kept.
- fp16 everywhere: DVE tensor_tensor runs in 2x mode for 2-byte packed
  dtypes (0.52 ns/elem vs 1.04 f32), and fp16's 10 mantissa bits keep
  the noise floor at ~1.7e-3 rel (bf16's 1.4e-2 was too close to the
  2e-2 gate).
- Host pre-gathers inputs into the exact SBUF slab layout
  [slab][part][step][lane] (lane = group*C + chunk) so every DMA is a
  fully contiguous 128-descriptor transfer; host scatters outputs back.
- Custom DVE op EKF_R1S fuses S = pp00+scale INTO the reciprocal
  (BITWISE_NOT exponent-flip seed + one Newton pass, 6/8 uop stages),
  so the innovation-variance reciprocal is ONE instruction.
- The whole Riccati recurrence stays DVE-local so the step-to-step
  dependency never crosses engines (cross-engine recurrences stall the
  in-order queues); Pool gets only slack-tolerant ops (pq, pp11, x1');
  ACT does the bulk sigmoid/scale derivation.
- x-part lags the P-part by XDELAY=4 steps so two independent
  dependency chains keep DVE at ~92% occupancy.
- NOTE: ns (steps/slab) smaller than 12 miscomputes on real HW
  (scheduler/clock-wait issue invisible to TimelineSim) - keep ns=12.
"""

import numpy as np

import concourse.bass as bass
import concourse.bacc as bacc
import concourse.mybir as mybir
import concourse.tile as tile
from concourse.dve_ops import RECIP_APPROX_FAST_CONSTS, RECIPROCAL_APPROX_FAST
from concourse.dve_spec import Spec, Src0, Src1, C0, C1, AluOp, Bin, lower
import concourse.dve_ops as dve_ops_mod
from concourse.dve_ops import DveOp, OPS
from concourse.dve_uop import DveOpSpec


def _register_dve_op(name, spec):
    for op in OPS:
        if op.name == name:
            return op
    shas = {}
    for ver in ("v3", "v4"):
        uops = lower(spec, ver=ver)
        shas[ver] = DveOpSpec(name=name, opcode=0, uops=uops, rd1_en=True).sha(ver)
    op = DveOp(name, spec, subdim=False, uops_sha=shas)
    OPS.append(op)
    dve_ops_mod.CUSTOM_DVE_SPECS[name] = spec
    dve_ops_mod._SUB_OPCODE_FOR_NAME[name] = (
        dve_ops_mod._CUSTOM_DVE_ROW_BASE + len(OPS) - 1
    )
    assert dve_ops_mod._SUB_OPCODE_FOR_NAME[name] < 0x20
    return op


def _ref_r1s(in0, in1, c0, c1, c2):
    import numpy as np
    x = np.asarray(in0, np.float32) + np.asarray(in1, np.float32)
    not_x = (~x.view(np.int32)).view(np.float32)
    y0 = not_x * np.float32(c0)
    return (y0 * (np.float32(c1) - x * y0)).astype(np.float32)


_x = Src0 + Src1
_nx = Bin(AluOp.BITWISE_NOT, _x, _x)
_y0f = _nx * C0
R1S = _register_dve_op(
    "EKF_R1S",
    Spec(body=_y0f * (C1 - _x * _y0f), reference=_ref_r1s),
)


def _ref_sq3(in0, in1, c0, c1, c2):
    import numpy as np
    a = np.asarray(in0, np.float32)
    return (a * a * np.asarray(in1, np.float32)).astype(np.float32)


SQ3 = _register_dve_op(
    "EKF_SQ3", Spec(body=Src0 * Src0 * Src1, reference=_ref_sq3)
)

F16 = mybir.dt.float16
F32 = mybir.dt.float32
ALU = mybir.AluOpType
ACT = mybir.ActivationFunctionType
PART = 128

# geometry
B, T = 8192, 2048
NCORES = 8
B_LOC = B // NCORES          # 1024
G = B_LOC // PART            # 8
W, L, C = 20, 52, 39         # warmup, chunk len, chunks; C*L + W == T
GC = G * C                   # 312 lanes per partition
STEPS = W + L                # 72
NS = 12                      # steps per slab
NSLAB = STEPS // NS          # 6
XDELAY = 4

assert C * L + W == T and NSLAB * NS == STEPS


def build_core_kernel(
    ns=6,
    xdelay=4,
    io_bufs=3,
    ost_bufs=3,
    stp_bufs=3,
    uk_extra=3,
    prefetch_back=5,
    t3_eng="V",
    p11n_eng="V",
    x0_eng="V",
    k1y_eng="V",
    y_eng="V",
    x1_eng="GP",
    pq_eng="GP",
    sq01_eng="V",
    p00n_eng="V",
    u_eng="V",
    p01n_eng="V",
    drop_sq01=False,
    mq_form=False,
    uy_eng="V",
    recip_act=False,
    bulk_nibble=False,
    chain_prio=None,
    recip_fuse=False,
    sq3_fuse=False,
    geo=(20, 52, 39),
):
    W_, L_, C_ = geo
    GC_ = G * C_
    STEPS_ = W_ + L_
    assert C_ * L_ + W_ == T
    NS = ns
    NSLAB = STEPS_ // NS
    XDELAY = xdelay
    assert NSLAB * NS == STEPS_
    NSGC = NS * GC_
    nc = bacc.Bacc("TRN2", target_bir_lowering=False, debug=False)
    z_h = nc.dram_tensor("z", [NSLAB, PART, NSGC], F16, kind="ExternalInput")
    h_h = nc.dram_tensor("h", [NSLAB, PART, NSGC], F16, kind="ExternalInput")
    v_h = nc.dram_tensor("v", [NSLAB, PART, NSGC], F16, kind="ExternalInput")
    out_h = nc.dram_tensor(
        "out", [NSLAB, PART, NS * 2 * GC], F16, kind="ExternalOutput"
    )

    rc = RECIP_APPROX_FAST_CONSTS

    def dram_ap(handle, si, width):
        return bass.AP(
            tensor=handle, offset=si * PART * width, ap=[[width, PART], [1, width]]
        )

    with tile.TileContext(nc) as tc:
        with (
            tc.tile_pool(name="io", bufs=io_bufs) as iop,
            tc.tile_pool(name="ost", bufs=ost_bufs) as ostp,
            tc.tile_pool(name="st", bufs=stp_bufs) as stp,
            tc.tile_pool(name="uk", bufs=XDELAY + uk_extra) as ukp,
            tc.tile_pool(name="ini", bufs=1) as inip,
        ):
            V = nc.vector
            GP = nc.gpsimd
            SC = nc.scalar
            E = {"V": V, "GP": GP}

            p00_i = inip.tile([PART, GC], F16, tag="p00i")
            p01_i = inip.tile([PART, GC], F16, tag="p01i")
            p11_i = inip.tile([PART, GC], F16, tag="p11i")
            x0_i = inip.tile([PART, GC], F16, tag="x0i")
            x1_i = inip.tile([PART, GC], F16, tag="x1i")
            bias_m5 = inip.tile([PART, 1], F32, tag="bm5")
            bias_m1 = inip.tile([PART, 1], F32, tag="bm1")
            GP.memset(bias_m5[:], -5.0)
            GP.memset(bias_m1[:], -1.0)
            GP.memset(p00_i[:], 1.0)
            GP.memset(p01_i[:], 0.0)
            GP.memset(p11_i[:], 1.0)
            # preload the ACT function table off the critical path
            SC.activation(
                x0_i[:][:, 0:1], bias_m5[:], ACT.Sigmoid,
                bias=bias_m5[:], scale=1.0,
            )

            slab_ctx = {}
            pprev = {}
            xprev = {}
            kctx = {}

            def load_slab_dma(si):
                z_sl = iop.tile([PART, NSGC], F16, tag="z")
                h_sl = iop.tile([PART, NSGC], F16, tag="h")
                v_sl = iop.tile([PART, NSGC], F16, tag="v")
                a_sl = iop.tile([PART, NSGC], F16, tag="a")
                s_sl = iop.tile([PART, NSGC], F16, tag="scl")
                q_sl = iop.tile([PART, NSGC], F16, tag="qq")
                o_sl = ostp.tile([PART, NS * 2 * GC], F16, tag="o")
                if si == 0:
                    fc = 2 * GC
                    for tl, hd in ((h_sl, h_h), (v_sl, v_h), (z_sl, z_h)):
                        nc.sync.dma_start(
                            tl[:][:, :fc],
                            bass.AP(
                                tensor=hd, offset=0,
                                ap=[[NSGC, PART], [1, fc]],
                            ),
                        )
                    for tl, hd in ((h_sl, h_h), (v_sl, v_h), (z_sl, z_h)):
                        nc.sync.dma_start(
                            tl[:][:, fc:],
                            bass.AP(
                                tensor=hd, offset=fc,
                                ap=[[NSGC, PART], [1, NSGC - fc]],
                            ),
                        )
                else:
                    nc.sync.dma_start(h_sl[:], dram_ap(h_h, si, NSGC))
                    nc.sync.dma_start(v_sl[:], dram_ap(v_h, si, NSGC))
                    nc.sync.dma_start(z_sl[:], dram_ap(z_h, si, NSGC))
                slab_ctx[si] = d = dict(
                    h_sl=h_sl, v_sl=v_sl, a_sl=a_sl, s_sl=s_sl, q_sl=q_sl,
                    zv=z_sl[:].rearrange("p (s gc) -> p s gc", s=NS),
                    av=a_sl[:].rearrange("p (s gc) -> p s gc", s=NS),
                    sv=s_sl[:].rearrange("p (s gc) -> p s gc", s=NS),
                    qv=q_sl[:].rearrange("p (s gc) -> p s gc", s=NS),
                    o_sl=o_sl,
                    ov=o_sl[:].rearrange(
                        "p (s two gc) -> p s two gc", s=NS, two=2
                    ),
                )
                if not bulk_nibble:
                    if si == 0:
                        bnds = [0, 2 * GC, 5 * GC, 8 * GC, NSGC]
                    else:
                        nk = NSGC // 4
                        bnds = [0, nk, 2 * nk, 3 * nk, NSGC]
                    for ci in range(4):
                        emit_bulk(si, slice(bnds[ci], bnds[ci + 1]))
                return slab_ctx[si]

            def _unused_emit_bulk_hook(si, rng):
                pass

            def emit_bulk(si, rng):
                """ACT derivation for one range of slab si:
                a = 0.5 + 0.5*sigmoid(10h-5); t = relu(100v-1);
                qq = 0.1t + 0.1; scl = t + 1."""
                sl = slab_ctx[si]
                cs = rng
                h_sl, v_sl = sl["h_sl"], sl["v_sl"]
                a_sl, s_sl, q_sl = sl["a_sl"], sl["s_sl"], sl["q_sl"]
                SC.activation(
                    a_sl[:][:, cs], h_sl[:][:, cs], ACT.Sigmoid,
                    bias=bias_m5[:], scale=10.0,
                )
                SC.activation(
                    a_sl[:][:, cs], a_sl[:][:, cs], ACT.Copy,
                    bias=0.5, scale=0.5,
                )
                SC.activation(
                    s_sl[:][:, cs], v_sl[:][:, cs], ACT.Relu,
                    bias=bias_m1[:], scale=100.0,
                )
                SC.activation(
                    q_sl[:][:, cs], s_sl[:][:, cs], ACT.Copy,
                    bias=0.1, scale=0.1,
                )
                SC.activation(
                    s_sl[:][:, cs], s_sl[:][:, cs], ACT.Copy,
                    bias=1.0, scale=1.0,
                )

            def emit_p_head(gs):
                si, s = divmod(gs, NS)
                sl = slab_ctx[si]
                A = sl["av"][:, s]
                QQ = sl["qv"][:, s]
                p00p, p01p, p11p = pprev["p00"], pprev["p01"], pprev["p11"]

                pp11 = stp.tile([PART, GC], F16, tag="pp11")
                if mq_form:
                    pq = None
                else:
                    pq = stp.tile([PART, GC], F16, tag="pq")
                    E[pq_eng].tensor_tensor(
                        out=pq[:], in0=QQ, in1=p00p, op=ALU.add
                    )
                E[pq_eng].tensor_tensor(out=pp11[:], in0=QQ, in1=p11p, op=ALU.add)

                t1 = stp.tile([PART, GC], F16, tag="t1")
                pp01 = stp.tile([PART, GC], F16, tag="pp01")
                g2 = stp.tile([PART, GC], F16, tag="g2")
                m = stp.tile([PART, GC], F16, tag="m")
                import contextlib
                hp = (tc.high_priority(offset=chain_prio)
                      if chain_prio else contextlib.nullcontext())
                with hp:
                    V.tensor_tensor(out=t1[:], in0=A, in1=p11p, op=ALU.mult)
                    V.tensor_tensor(
                        out=pp01[:], in0=p01p, in1=t1[:], op=ALU.add
                    )
                    V.tensor_tensor(
                        out=g2[:], in0=pp01[:], in1=p01p, op=ALU.add
                    )
                    V.tensor_tensor(out=m[:], in0=A, in1=g2[:], op=ALU.mult)
                return dict(sl=sl, s=s, pq=pq, pp11=pp11, pp01=pp01, m=m,
                            p00p=p00p)

            def emit_p_tail(gs, h):
                import contextlib
                hp = (tc.high_priority(offset=chain_prio)
                      if chain_prio else contextlib.nullcontext())
                sl, s = h["sl"], h["s"]
                SCL = sl["sv"][:, s]
                pq, pp11, pp01, m = (
                    h["pq"], h["pp11"], h["pp01"], h["m"]
                )
                pp00 = stp.tile([PART, GC], F16, tag="pp00")
                S = (None if recip_fuse
                     else stp.tile([PART, GC], F16, tag="S"))
                r = stp.tile([PART, GC], F16, tag="r")
                u = ukp.tile([PART, GC], F16, tag="u")
                k1 = ukp.tile([PART, GC], F16, tag="k1")
                p00n = stp.tile([PART, GC], F16, tag="p00")
                p01n = stp.tile([PART, GC], F16, tag="p01")
                t3 = stp.tile([PART, GC], F16, tag="t3")
                p11n = stp.tile([PART, GC], F16, tag="p11")
                with hp:
                    if mq_form:
                        QQ = sl["qv"][:, s]
                        mq = stp.tile([PART, GC], F16, tag="mq")
                        V.tensor_tensor(
                            out=mq[:], in0=m[:], in1=QQ, op=ALU.add
                        )
                        V.tensor_tensor(
                            out=pp00[:], in0=mq[:], in1=h["p00p"], op=ALU.add
                        )
                    else:
                        V.tensor_tensor(
                            out=pp00[:], in0=pq[:], in1=m[:], op=ALU.add
                        )
                    if not recip_fuse:
                        V.tensor_tensor(
                            out=S[:], in0=pp00[:], in1=SCL, op=ALU.add
                        )
                if recip_fuse:
                    V._custom_dve(
                        R1S, out=r[:], in0=pp00[:], in1=SCL,
                        s0=rc["s0"], s1=rc["s1"],
                    )
                elif recip_act:
                    SC.add_instruction(mybir.InstActivation(
                        name=nc.get_next_instruction_name(),
                        func=ACT.Reciprocal,
                        ins=[
                            SC.lower_ap(S[:]),
                            mybir.ImmediateValue(dtype=F32, value=0.0),
                            mybir.ImmediateValue(dtype=F32, value=1.0),
                            mybir.ImmediateValue(dtype=F32, value=0.0),
                        ],
                        outs=[SC.lower_ap(r[:])],
                    ))
                else:
                    V._custom_dve(
                        RECIPROCAL_APPROX_FAST, out=r[:], in0=S[:],
                        s0=rc["s0"], s1=rc["s1"], imm2=rc["imm2"],
                    )
                E[u_eng].tensor_tensor(
                    out=u[:], in0=SCL, in1=r[:], op=ALU.mult
                )
                V.tensor_tensor(out=k1[:], in0=r[:], in1=pp01[:], op=ALU.mult)
                E[p00n_eng].tensor_tensor(
                    out=p00n[:], in0=u[:], in1=pp00[:], op=ALU.mult
                )
                E[p01n_eng].tensor_tensor(
                    out=p01n[:], in0=SCL, in1=k1[:], op=ALU.mult
                )
                if sq3_fuse:
                    V._custom_dve(SQ3, out=t3[:], in0=pp01[:], in1=r[:])
                elif drop_sq01:
                    E[t3_eng].tensor_tensor(
                        out=t3[:], in0=k1[:], in1=pp01[:], op=ALU.mult
                    )
                else:
                    sq01 = stp.tile([PART, GC], F16, tag="sq01")
                    E[sq01_eng].tensor_tensor(
                        out=sq01[:], in0=pp01[:], in1=pp01[:], op=ALU.mult
                    )
                    E[t3_eng].tensor_tensor(
                        out=t3[:], in0=sq01[:], in1=r[:], op=ALU.mult
                    )
                E[p11n_eng].tensor_tensor(
                    out=p11n[:], in0=pp11[:], in1=t3[:], op=ALU.subtract
                )
                pprev.update(p00=p00n[:], p01=p01n[:], p11=p11n[:])
                kctx[gs] = (u, k1)

            def emit_x(gs):
                si, s = divmod(gs, NS)
                sl = slab_ctx[si]
                Z = sl["zv"][:, s]
                A = sl["av"][:, s]
                ov = sl["ov"]
                u, k1 = kctx.pop(gs)
                x0p, x1p = xprev["x0"], xprev["x1"]

                t4 = stp.tile([PART, GC], F16, tag="t4")
                xp = stp.tile([PART, GC], F16, tag="xp")
                y = stp.tile([PART, GC], F16, tag="y")
                uy = stp.tile([PART, GC], F16, tag="uy")
                k1y = stp.tile([PART, GC], F16, tag="k1y")
                V.tensor_tensor(out=t4[:], in0=A, in1=x1p, op=ALU.mult)
                V.tensor_tensor(out=xp[:], in0=x0p, in1=t4[:], op=ALU.add)
                E[y_eng].tensor_tensor(
                    out=y[:], in0=Z, in1=xp[:], op=ALU.subtract
                )
                E[uy_eng].tensor_tensor(
                    out=uy[:], in0=u[:], in1=y[:], op=ALU.mult
                )
                E[x0_eng].tensor_tensor(
                    out=ov[:, s, 0], in0=Z, in1=uy[:], op=ALU.subtract
                )
                E[k1y_eng].tensor_tensor(
                    out=k1y[:], in0=k1[:], in1=y[:], op=ALU.mult
                )
                E[x1_eng].tensor_tensor(
                    out=ov[:, s, 1], in0=x1p, in1=k1y[:], op=ALU.add
                )
                xprev.update(x0=ov[:, s, 0], x1=ov[:, s, 1])

                if si == NSLAB - 1:
                    half = (NS // 2) * 2 * GC
                    if s == NS // 2 - 1:
                        nc.sync.dma_start(
                            bass.AP(
                                tensor=out_h,
                                offset=si * PART * NS * 2 * GC,
                                ap=[[NS * 2 * GC, PART], [1, half]],
                            ),
                            sl["o_sl"][:][:, :half],
                        )
                    elif s == NS - 1:
                        nc.sync.dma_start(
                            bass.AP(
                                tensor=out_h,
                                offset=si * PART * NS * 2 * GC + half,
                                ap=[[NS * 2 * GC, PART], [1, half]],
                            ),
                            sl["o_sl"][:][:, half:],
                        )
                elif s == NS - 1:
                    nc.sync.dma_start(
                        dram_ap(out_h, si, NS * 2 * GC), sl["o_sl"][:]
                    )

            for gs in range(STEPS + XDELAY):
                if gs < STEPS:
                    si, s = divmod(gs, NS)
                    if gs == 0:
                        ctx = load_slab_dma(0)
                        if bulk_nibble:
                            nkr0 = NSGC // NS
                            emit_bulk(0, slice(0, 2 * nkr0))
                        zv = ctx["zv"]
                        V.tensor_copy(x0_i[:], zv[:, 0])
                        V.tensor_tensor(
                            out=x1_i[:], in0=zv[:, 1], in1=zv[:, 0],
                            op=ALU.subtract,
                        )
                        pprev.update(
                            p00=p00_i[:], p01=p01_i[:], p11=p11_i[:]
                        )
                        xprev.update(x0=x0_i[:], x1=x1_i[:])
                    if s == max(0, NS - prefetch_back) and si + 1 < NSLAB:
                        load_slab_dma(si + 1)
                    h = emit_p_head(gs)
                    if gs - XDELAY >= 0:
                        emit_x(gs - XDELAY)
                    emit_p_tail(gs, h)
                    if bulk_nibble:
                        nkr = NSGC // NS
                        if si == 0 and 0 <= gs < NS - 2:
                            emit_bulk(0, slice((gs + 2) * nkr, (gs + 3) * nkr))
                        pf = max(0, NS - prefetch_back)
                        if si + 1 < NSLAB and s >= pf:
                            left = NS - pf
                            r0 = (s - pf) * NS // left
                            r1 = (s - pf + 1) * NS // left
                            if r1 > r0:
                                emit_bulk(
                                    si + 1, slice(r0 * nkr, r1 * nkr)
                                )
                else:
                    emit_x(gs - XDELAY)
    nc.compile()
    return nc


_nc_cache = {}


def _get_nc():
    if "nc" not in _nc_cache:
        # best verified config: chain-local Riccati on DVE + fused-S
        # 1-Newton reciprocal custom op; 341.4us/core, rel err 4.2e-3
        _nc_cache["nc"] = build_core_kernel(
            ns=12, io_bufs=2, recip_fuse=True
        )
    return _nc_cache["nc"]


# host-side gather/scatter index: col of (step gs, chunk c) = c*L + gs
_COLS = (np.arange(C)[None, :] * L + np.arange(STEPS)[:, None])  # [STEPS, C]


def _stage_input(arr):
    """[B_LOC, T] f32 -> [NSLAB, PART, NS*GC] f16 in slab layout."""
    xf = arr.astype(np.float16)
    g3 = xf.reshape(G, PART, T)[:, :, _COLS]          # [G, P, STEPS, C]
    g4 = np.transpose(g3, (2, 1, 0, 3))                # [STEPS, P, G, C]
    st = g4.reshape(NSLAB, NS, PART, GC)
    st = np.transpose(st, (0, 2, 1, 3)).reshape(NSLAB, PART, NS * GC)
    return np.ascontiguousarray(st)


def _unstage_output(dev_out):
    """[NSLAB, PART, NS*2*GC] f16 -> [B_LOC, T, 2] f32."""
    o = np.asarray(dev_out).reshape(NSLAB, PART, NS, 2, G, C)
    o = np.transpose(o, (4, 1, 0, 2, 5, 3))            # [G, P, NSLAB, NS, C, 2]
    o = o.reshape(B_LOC, STEPS, C, 2).astype(np.float32)
    res = np.empty((B_LOC, T, 2), np.float32)
    res[:, _COLS[:, 0], :] = o[:, :, 0, :]
    res[:, _COLS[W:, 1:], :] = o[:, W:, 1:, :]
    return res


def kernel(price: np.ndarray, hurst: np.ndarray, vol_sigma: np.ndarray) -> np.ndarray:
    from concourse import bass_utils

    price = np.ascontiguousarray(price, dtype=np.float32)
    hurst = np.ascontiguousarray(hurst, dtype=np.float32)
    vol_sigma = np.ascontiguousarray(vol_sigma, dtype=np.float32)
    nc = _get_nc()
    in_maps = []
    for k in range(NCORES):
        sl = slice(k * B_LOC, (k + 1) * B_LOC)
        in_maps.append(
            {
                "z": _stage_input(price[sl]),
                "h": _stage_input(hurst[sl]),
                "v": _stage_input(vol_sigma[sl]),
            }
        )
    res = bass_utils.run_bass_kernel_spmd(
        nc, in_maps, core_ids=list(range(NCORES))
    )
    return np.concatenate(
        [_unstage_output(r["out"]) for r in res.results], axis=0
    )


# revision 38
# speedup vs baseline: 1.0428x; 1.0068x over previous
"""Trainium2 Bass kernel for the batched differentiable EKF.

B=8192 rows x T=2048 sequential EKF steps (2-state KF, scalar obs).
Output [B, T, 2] f32.

Design (final, device-verified 341.4us/core, rel err 4.2e-3):
- Data parallel: 1024 rows/core over 8 cores; rows -> 8 groups x 128
  partitions.
- Time parallel per core: T split into C=39 chunks of L=52 steps with a
  W=20-step warmup from a cold init (x=[z,dz], P=I). Chunk 0's warmup is
  the true filter start, so its warmup outputs are kept.
- fp16 everywhere: DVE tensor_tensor runs in 2x mode for 2-byte packed
  dtypes (0.52 ns/elem vs 1.04 f32), and fp16's 10 mantissa bits keep
  the noise floor at ~1.7e-3 rel (bf16's 1.4e-2 was too close to the
  2e-2 gate).
- Host pre-gathers inputs into the exact SBUF slab layout
  [slab][part][step][lane] (lane = group*C + chunk) so every DMA is a
  fully contiguous 128-descriptor transfer; host scatters outputs back.
- Custom DVE op EKF_R1S fuses S = pp00+scale INTO the reciprocal
  (BITWISE_NOT exponent-flip seed + one Newton pass, 6/8 uop stages),
  so the innovation-variance reciprocal is ONE instruction.
- The whole Riccati recurrence stays DVE-local so the step-to-step
  dependency never crosses engines (cross-engine recurrences stall the
  in-order queues); Pool gets only slack-tolerant ops (pq, pp11, x1');
  ACT does the bulk sigmoid/scale derivation.
- x-part lags the P-part by XDELAY=4 steps so two independent
  dependency chains keep DVE at ~92% occupancy.
- NOTE: ns (steps/slab) smaller than 12 miscomputes on real HW
  (scheduler/clock-wait issue invisible to TimelineSim) - keep ns=12.
"""

import numpy as np

import concourse.bass as bass
import concourse.bacc as bacc
import concourse.mybir as mybir
import concourse.tile as tile
from concourse.dve_ops import RECIP_APPROX_FAST_CONSTS, RECIPROCAL_APPROX_FAST
from concourse.dve_spec import Spec, Src0, Src1, C0, C1, AluOp, Bin, lower
import concourse.dve_ops as dve_ops_mod
from concourse.dve_ops import DveOp, OPS
from concourse.dve_uop import DveOpSpec


def _register_dve_op(name, spec):
    for op in OPS:
        if op.name == name:
            return op
    shas = {}
    for ver in ("v3", "v4"):
        uops = lower(spec, ver=ver)
        shas[ver] = DveOpSpec(name=name, opcode=0, uops=uops, rd1_en=True).sha(ver)
    op = DveOp(name, spec, subdim=False, uops_sha=shas)
    OPS.append(op)
    dve_ops_mod.CUSTOM_DVE_SPECS[name] = spec
    dve_ops_mod._SUB_OPCODE_FOR_NAME[name] = (
        dve_ops_mod._CUSTOM_DVE_ROW_BASE + len(OPS) - 1
    )
    assert dve_ops_mod._SUB_OPCODE_FOR_NAME[name] < 0x20
    return op


def _ref_r1s(in0, in1, c0, c1, c2):
    import numpy as np
    x = np.asarray(in0, np.float32) + np.asarray(in1, np.float32)
    not_x = (~x.view(np.int32)).view(np.float32)
    y0 = not_x * np.float32(c0)
    return (y0 * (np.float32(c1) - x * y0)).astype(np.float32)


_x = Src0 + Src1
_nx = Bin(AluOp.BITWISE_NOT, _x, _x)
_y0f = _nx * C0
R1S = _register_dve_op(
    "EKF_R1S",
    Spec(body=_y0f * (C1 - _x * _y0f), reference=_ref_r1s),
)


def _ref_sq3(in0, in1, c0, c1, c2):
    import numpy as np
    a = np.asarray(in0, np.float32)
    return (a * a * np.asarray(in1, np.float32)).astype(np.float32)


SQ3 = _register_dve_op(
    "EKF_SQ3", Spec(body=Src0 * Src0 * Src1, reference=_ref_sq3)
)

F16 = mybir.dt.float16
F32 = mybir.dt.float32
ALU = mybir.AluOpType
ACT = mybir.ActivationFunctionType
PART = 128

# geometry
B, T = 8192, 2048
NCORES = 8
B_LOC = B // NCORES          # 1024
G = B_LOC // PART            # 8
W, L, C = 20, 52, 39         # warmup, chunk len, chunks; C*L + W == T
GC = G * C                   # 312 lanes per partition
STEPS = W + L                # 72
NS = 12                      # steps per slab
NSLAB = STEPS // NS          # 6
XDELAY = 4

assert C * L + W == T and NSLAB * NS == STEPS


def build_core_kernel(
    ns=6,
    xdelay=4,
    io_bufs=3,
    ost_bufs=3,
    stp_bufs=3,
    uk_extra=3,
    prefetch_back=5,
    t3_eng="V",
    p11n_eng="V",
    x0_eng="V",
    k1y_eng="V",
    y_eng="V",
    x1_eng="GP",
    pq_eng="GP",
    sq01_eng="V",
    p00n_eng="V",
    u_eng="V",
    p01n_eng="V",
    drop_sq01=False,
    mq_form=False,
    uy_eng="V",
    recip_act=False,
    bulk_nibble=False,
    chain_prio=None,
    recip_fuse=False,
    sq3_fuse=False,
    geo=(20, 52, 39),
):
    W_, L_, C_ = geo
    GC_ = G * C_
    STEPS_ = W_ + L_
    assert C_ * L_ + W_ == T
    NS = ns
    NSLAB = STEPS_ // NS
    XDELAY = xdelay
    assert NSLAB * NS == STEPS_
    NSGC = NS * GC_
    nc = bacc.Bacc("TRN2", target_bir_lowering=False, debug=False)
    z_h = nc.dram_tensor("z", [NSLAB, PART, NSGC], F16, kind="ExternalInput")
    h_h = nc.dram_tensor("h", [NSLAB, PART, NSGC], F16, kind="ExternalInput")
    v_h = nc.dram_tensor("v", [NSLAB, PART, NSGC], F16, kind="ExternalInput")
    out_h = nc.dram_tensor(
        "out", [NSLAB, PART, NS * 2 * GC], F16, kind="ExternalOutput"
    )

    rc = RECIP_APPROX_FAST_CONSTS

    def dram_ap(handle, si, width):
        return bass.AP(
            tensor=handle, offset=si * PART * width, ap=[[width, PART], [1, width]]
        )

    with tile.TileContext(nc) as tc:
        with (
            tc.tile_pool(name="io", bufs=io_bufs) as iop,
            tc.tile_pool(name="ost", bufs=ost_bufs) as ostp,
            tc.tile_pool(name="st", bufs=stp_bufs) as stp,
            tc.tile_pool(name="uk", bufs=XDELAY + uk_extra) as ukp,
            tc.tile_pool(name="ini", bufs=1) as inip,
        ):
            V = nc.vector
            GP = nc.gpsimd
            SC = nc.scalar
            E = {"V": V, "GP": GP}

            p00_i = inip.tile([PART, GC], F16, tag="p00i")
            p01_i = inip.tile([PART, GC], F16, tag="p01i")
            p11_i = inip.tile([PART, GC], F16, tag="p11i")
            x0_i = inip.tile([PART, GC], F16, tag="x0i")
            x1_i = inip.tile([PART, GC], F16, tag="x1i")
            bias_m5 = inip.tile([PART, 1], F32, tag="bm5")
            bias_m1 = inip.tile([PART, 1], F32, tag="bm1")
            GP.memset(bias_m5[:], -5.0)
            GP.memset(bias_m1[:], -1.0)
            GP.memset(p00_i[:], 1.0)
            GP.memset(p01_i[:], 0.0)
            GP.memset(p11_i[:], 1.0)
            # preload the ACT function table off the critical path
            SC.activation(
                x0_i[:][:, 0:1], bias_m5[:], ACT.Sigmoid,
                bias=bias_m5[:], scale=1.0,
            )

            slab_ctx = {}
            pprev = {}
            xprev = {}
            kctx = {}

            def load_slab_dma(si):
                z_sl = iop.tile([PART, NSGC], F16, tag="z")
                h_sl = iop.tile([PART, NSGC], F16, tag="h")
                v_sl = iop.tile([PART, NSGC], F16, tag="v")
                a_sl = iop.tile([PART, NSGC], F16, tag="a")
                s_sl = iop.tile([PART, NSGC], F16, tag="scl")
                q_sl = iop.tile([PART, NSGC], F16, tag="qq")
                o_sl = ostp.tile([PART, NS * 2 * GC], F16, tag="o")
                if si == 0:
                    fc = 2 * GC
                    for tl, hd in ((h_sl, h_h), (v_sl, v_h), (z_sl, z_h)):
                        nc.sync.dma_start(
                            tl[:][:, :fc],
                            bass.AP(
                                tensor=hd, offset=0,
                                ap=[[NSGC, PART], [1, fc]],
                            ),
                        )
                    for tl, hd in ((h_sl, h_h), (v_sl, v_h), (z_sl, z_h)):
                        nc.sync.dma_start(
                            tl[:][:, fc:],
                            bass.AP(
                                tensor=hd, offset=fc,
                                ap=[[NSGC, PART], [1, NSGC - fc]],
                            ),
                        )
                else:
                    nc.sync.dma_start(h_sl[:], dram_ap(h_h, si, NSGC))
                    nc.sync.dma_start(v_sl[:], dram_ap(v_h, si, NSGC))
                    nc.sync.dma_start(z_sl[:], dram_ap(z_h, si, NSGC))
                slab_ctx[si] = d = dict(
                    h_sl=h_sl, v_sl=v_sl, a_sl=a_sl, s_sl=s_sl, q_sl=q_sl,
                    zv=z_sl[:].rearrange("p (s gc) -> p s gc", s=NS),
                    av=a_sl[:].rearrange("p (s gc) -> p s gc", s=NS),
                    sv=s_sl[:].rearrange("p (s gc) -> p s gc", s=NS),
                    qv=q_sl[:].rearrange("p (s gc) -> p s gc", s=NS),
                    o_sl=o_sl,
                    ov=o_sl[:].rearrange(
                        "p (s two gc) -> p s two gc", s=NS, two=2
                    ),
                )
                if not bulk_nibble:
                    if si == 0:
                        bnds = [0, 2 * GC, 5 * GC, 8 * GC, NSGC]
                    else:
                        nk = NSGC // 4
                        bnds = [0, nk, 2 * nk, 3 * nk, NSGC]
                    for ci in range(4):
                        emit_bulk(si, slice(bnds[ci], bnds[ci + 1]))
                return slab_ctx[si]

            def _unused_emit_bulk_hook(si, rng):
                pass

            def emit_bulk(si, rng):
                """ACT derivation for one range of slab si:
                a = 0.5 + 0.5*sigmoid(10h-5); t = relu(100v-1);
                qq = 0.1t + 0.1; scl = t + 1."""
                sl = slab_ctx[si]
                cs = rng
                h_sl, v_sl = sl["h_sl"], sl["v_sl"]
                a_sl, s_sl, q_sl = sl["a_sl"], sl["s_sl"], sl["q_sl"]
                SC.activation(
                    a_sl[:][:, cs], h_sl[:][:, cs], ACT.Sigmoid,
                    bias=bias_m5[:], scale=10.0,
                )
                SC.activation(
                    a_sl[:][:, cs], a_sl[:][:, cs], ACT.Copy,
                    bias=0.5, scale=0.5,
                )
                SC.activation(
                    s_sl[:][:, cs], v_sl[:][:, cs], ACT.Relu,
                    bias=bias_m1[:], scale=100.0,
                )
                SC.activation(
                    q_sl[:][:, cs], s_sl[:][:, cs], ACT.Copy,
                    bias=0.1, scale=0.1,
                )
                SC.activation(
                    s_sl[:][:, cs], s_sl[:][:, cs], ACT.Copy,
                    bias=1.0, scale=1.0,
                )

            def emit_p_head(gs):
                si, s = divmod(gs, NS)
                sl = slab_ctx[si]
                A = sl["av"][:, s]
                QQ = sl["qv"][:, s]
                p00p, p01p, p11p = pprev["p00"], pprev["p01"], pprev["p11"]

                pp11 = stp.tile([PART, GC], F16, tag="pp11")
                if mq_form:
                    pq = None
                else:
                    pq = stp.tile([PART, GC], F16, tag="pq")
                    E[pq_eng].tensor_tensor(
                        out=pq[:], in0=QQ, in1=p00p, op=ALU.add
                    )
                E[pq_eng].tensor_tensor(out=pp11[:], in0=QQ, in1=p11p, op=ALU.add)

                t1 = stp.tile([PART, GC], F16, tag="t1")
                pp01 = stp.tile([PART, GC], F16, tag="pp01")
                g2 = stp.tile([PART, GC], F16, tag="g2")
                m = stp.tile([PART, GC], F16, tag="m")
                import contextlib
                hp = (tc.high_priority(offset=chain_prio)
                      if chain_prio else contextlib.nullcontext())
                with hp:
                    V.tensor_tensor(out=t1[:], in0=A, in1=p11p, op=ALU.mult)
                    V.tensor_tensor(
                        out=pp01[:], in0=p01p, in1=t1[:], op=ALU.add
                    )
                    V.tensor_tensor(
                        out=g2[:], in0=pp01[:], in1=p01p, op=ALU.add
                    )
                    V.tensor_tensor(out=m[:], in0=A, in1=g2[:], op=ALU.mult)
                return dict(sl=sl, s=s, pq=pq, pp11=pp11, pp01=pp01, m=m,
                            p00p=p00p)

            def emit_p_tail(gs, h):
                import contextlib
                hp = (tc.high_priority(offset=chain_prio)
                      if chain_prio else contextlib.nullcontext())
                sl, s = h["sl"], h["s"]
                SCL = sl["sv"][:, s]
                pq, pp11, pp01, m = (
                    h["pq"], h["pp11"], h["pp01"], h["m"]
                )
                pp00 = stp.tile([PART, GC], F16, tag="pp00")
                S = (None if recip_fuse
                     else stp.tile([PART, GC], F16, tag="S"))
                r = stp.tile([PART, GC], F16, tag="r")
                u = ukp.tile([PART, GC], F16, tag="u")
                k1 = ukp.tile([PART, GC], F16, tag="k1")
                p00n = stp.tile([PART, GC], F16, tag="p00")
                p01n = stp.tile([PART, GC], F16, tag="p01")
                t3 = stp.tile([PART, GC], F16, tag="t3")
                p11n = stp.tile([PART, GC], F16, tag="p11")
                with hp:
                    if mq_form:
                        QQ = sl["qv"][:, s]
                        mq = stp.tile([PART, GC], F16, tag="mq")
                        V.tensor_tensor(
                            out=mq[:], in0=m[:], in1=QQ, op=ALU.add
                        )
                        V.tensor_tensor(
                            out=pp00[:], in0=mq[:], in1=h["p00p"], op=ALU.add
                        )
                    else:
                        V.tensor_tensor(
                            out=pp00[:], in0=pq[:], in1=m[:], op=ALU.add
                        )
                    if not recip_fuse:
                        V.tensor_tensor(
                            out=S[:], in0=pp00[:], in1=SCL, op=ALU.add
                        )
                if recip_fuse:
                    V._custom_dve(
                        R1S, out=r[:], in0=pp00[:], in1=SCL,
                        s0=rc["s0"], s1=rc["s1"],
                    )
                elif recip_act:
                    SC.add_instruction(mybir.InstActivation(
                        name=nc.get_next_instruction_name(),
                        func=ACT.Reciprocal,
                        ins=[
                            SC.lower_ap(S[:]),
                            mybir.ImmediateValue(dtype=F32, value=0.0),
                            mybir.ImmediateValue(dtype=F32, value=1.0),
                            mybir.ImmediateValue(dtype=F32, value=0.0),
                        ],
                        outs=[SC.lower_ap(r[:])],
                    ))
                else:
                    V._custom_dve(
                        RECIPROCAL_APPROX_FAST, out=r[:], in0=S[:],
                        s0=rc["s0"], s1=rc["s1"], imm2=rc["imm2"],
                    )
                E[u_eng].tensor_tensor(
                    out=u[:], in0=SCL, in1=r[:], op=ALU.mult
                )
                V.tensor_tensor(out=k1[:], in0=r[:], in1=pp01[:], op=ALU.mult)
                last = gs == STEPS - 1
                if not last:
                    E[p00n_eng].tensor_tensor(
                        out=p00n[:], in0=u[:], in1=pp00[:], op=ALU.mult
                    )
                    E[p01n_eng].tensor_tensor(
                        out=p01n[:], in0=SCL, in1=k1[:], op=ALU.mult
                    )
                if last:
                    pprev.update(p00=None, p01=None, p11=None)
                    kctx[gs] = (u, k1)
                    return
                if sq3_fuse:
                    V._custom_dve(SQ3, out=t3[:], in0=pp01[:], in1=r[:])
                elif drop_sq01:
                    E[t3_eng].tensor_tensor(
                        out=t3[:], in0=k1[:], in1=pp01[:], op=ALU.mult
                    )
                else:
                    sq01 = stp.tile([PART, GC], F16, tag="sq01")
                    E[sq01_eng].tensor_tensor(
                        out=sq01[:], in0=pp01[:], in1=pp01[:], op=ALU.mult
                    )
                    E[t3_eng].tensor_tensor(
                        out=t3[:], in0=sq01[:], in1=r[:], op=ALU.mult
                    )
                E[p11n_eng].tensor_tensor(
                    out=p11n[:], in0=pp11[:], in1=t3[:], op=ALU.subtract
                )
                pprev.update(p00=p00n[:], p01=p01n[:], p11=p11n[:])
                kctx[gs] = (u, k1)

            def emit_x(gs):
                si, s = divmod(gs, NS)
                sl = slab_ctx[si]
                Z = sl["zv"][:, s]
                A = sl["av"][:, s]
                ov = sl["ov"]
                u, k1 = kctx.pop(gs)
                x0p, x1p = xprev["x0"], xprev["x1"]

                t4 = stp.tile([PART, GC], F16, tag="t4")
                xp = stp.tile([PART, GC], F16, tag="xp")
                y = stp.tile([PART, GC], F16, tag="y")
                uy = stp.tile([PART, GC], F16, tag="uy")
                k1y = stp.tile([PART, GC], F16, tag="k1y")
                V.tensor_tensor(out=t4[:], in0=A, in1=x1p, op=ALU.mult)
                V.tensor_tensor(out=xp[:], in0=x0p, in1=t4[:], op=ALU.add)
                E[y_eng].tensor_tensor(
                    out=y[:], in0=Z, in1=xp[:], op=ALU.subtract
                )
                E[uy_eng].tensor_tensor(
                    out=uy[:], in0=u[:], in1=y[:], op=ALU.mult
                )
                E[x0_eng].tensor_tensor(
                    out=ov[:, s, 0], in0=Z, in1=uy[:], op=ALU.subtract
                )
                E[k1y_eng].tensor_tensor(
                    out=k1y[:], in0=k1[:], in1=y[:], op=ALU.mult
                )
                E[x1_eng].tensor_tensor(
                    out=ov[:, s, 1], in0=x1p, in1=k1y[:], op=ALU.add
                )
                xprev.update(x0=ov[:, s, 0], x1=ov[:, s, 1])

                if si == NSLAB - 1:
                    qn = NS // 4
                    if (s + 1) % qn == 0:
                        qi = (s + 1) // qn - 1
                        qw = qn * 2 * GC
                        nc.sync.dma_start(
                            bass.AP(
                                tensor=out_h,
                                offset=si * PART * NS * 2 * GC + qi * qw,
                                ap=[[NS * 2 * GC, PART], [1, qw]],
                            ),
                            sl["o_sl"][:][:, qi * qw:(qi + 1) * qw],
                        )
                elif s == NS - 1:
                    nc.sync.dma_start(
                        dram_ap(out_h, si, NS * 2 * GC), sl["o_sl"][:]
                    )

            for gs in range(STEPS + XDELAY):
                if gs < STEPS:
                    si, s = divmod(gs, NS)
                    if gs == 0:
                        ctx = load_slab_dma(0)
                        if bulk_nibble:
                            nkr0 = NSGC // NS
                            emit_bulk(0, slice(0, 2 * nkr0))
                        zv = ctx["zv"]
                        V.tensor_copy(x0_i[:], zv[:, 0])
                        V.tensor_tensor(
                            out=x1_i[:], in0=zv[:, 1], in1=zv[:, 0],
                            op=ALU.subtract,
                        )
                        pprev.update(
                            p00=p00_i[:], p01=p01_i[:], p11=p11_i[:]
                        )
                        xprev.update(x0=x0_i[:], x1=x1_i[:])
                    if s == max(0, NS - prefetch_back) and si + 1 < NSLAB:
                        load_slab_dma(si + 1)
                    h = emit_p_head(gs)
                    if gs - XDELAY >= 0:
                        emit_x(gs - XDELAY)
                    emit_p_tail(gs, h)
                    if bulk_nibble:
                        nkr = NSGC // NS
                        if si == 0 and 0 <= gs < NS - 2:
                            emit_bulk(0, slice((gs + 2) * nkr, (gs + 3) * nkr))
                        pf = max(0, NS - prefetch_back)
                        if si + 1 < NSLAB and s >= pf:
                            left = NS - pf
                            r0 = (s - pf) * NS // left
                            r1 = (s - pf + 1) * NS // left
                            if r1 > r0:
                                emit_bulk(
                                    si + 1, slice(r0 * nkr, r1 * nkr)
                                )
                else:
                    emit_x(gs - XDELAY)
    nc.compile()
    return nc


_nc_cache = {}


def _get_nc():
    if "nc" not in _nc_cache:
        # best verified config: chain-local Riccati on DVE + fused-S
        # 1-Newton reciprocal custom op; 341.4us/core, rel err 4.2e-3
        _nc_cache["nc"] = build_core_kernel(
            ns=12, io_bufs=2, recip_fuse=True
        )
    return _nc_cache["nc"]


# host-side gather/scatter index: col of (step gs, chunk c) = c*L + gs
_COLS = (np.arange(C)[None, :] * L + np.arange(STEPS)[:, None])  # [STEPS, C]


def _stage_input(arr):
    """[B_LOC, T] f32 -> [NSLAB, PART, NS*GC] f16 in slab layout."""
    xf = arr.astype(np.float16)
    g3 = xf.reshape(G, PART, T)[:, :, _COLS]          # [G, P, STEPS, C]
    g4 = np.transpose(g3, (2, 1, 0, 3))                # [STEPS, P, G, C]
    st = g4.reshape(NSLAB, NS, PART, GC)
    st = np.transpose(st, (0, 2, 1, 3)).reshape(NSLAB, PART, NS * GC)
    return np.ascontiguousarray(st)


def _unstage_output(dev_out):
    """[NSLAB, PART, NS*2*GC] f16 -> [B_LOC, T, 2] f32."""
    o = np.asarray(dev_out).reshape(NSLAB, PART, NS, 2, G, C)
    o = np.transpose(o, (4, 1, 0, 2, 5, 3))            # [G, P, NSLAB, NS, C, 2]
    o = o.reshape(B_LOC, STEPS, C, 2).astype(np.float32)
    res = np.empty((B_LOC, T, 2), np.float32)
    res[:, _COLS[:, 0], :] = o[:, :, 0, :]
    res[:, _COLS[W:, 1:], :] = o[:, W:, 1:, :]
    return res


def kernel(price: np.ndarray, hurst: np.ndarray, vol_sigma: np.ndarray) -> np.ndarray:
    from concourse import bass_utils

    price = np.ascontiguousarray(price, dtype=np.float32)
    hurst = np.ascontiguousarray(hurst, dtype=np.float32)
    vol_sigma = np.ascontiguousarray(vol_sigma, dtype=np.float32)
    nc = _get_nc()
    in_maps = []
    for k in range(NCORES):
        sl = slice(k * B_LOC, (k + 1) * B_LOC)
        in_maps.append(
            {
                "z": _stage_input(price[sl]),
                "h": _stage_input(hurst[sl]),
                "v": _stage_input(vol_sigma[sl]),
            }
        )
    res = bass_utils.run_bass_kernel_spmd(
        nc, in_maps, core_ids=list(range(NCORES))
    )
    return np.concatenate(
        [_unstage_output(r["out"]) for r in res.results], axis=0
    )
